# revision 12
# baseline (speedup 1.0000x reference)
"""Causal self-attention (B=4, T=2048, D=1024, H=16) on 8 TRN2 NeuronCores.

Sharding: data parallel over batch (4 batches x 2 core-pairs) and tensor
parallel over heads (8 heads per core). Each core:
  - projects its batch's tokens to Q/K/V for its 8 heads (fp16 matmuls,
    fp32 PSUM accumulation),
  - runs causal flash-style attention in "scores transposed" orientation
    (S_T[key, query] = K_feat.T-stationary @ Q_feat-moving) so the softmax
    probabilities come out in the right orientation to be the stationary
    operand of P@V with no transpose,
  - softmax without max-subtraction (scores ~ N(0,1); fp32 exp range is
    ample) with denominators from an extra ones-column appended to V,
  - pairwise AllGather exchanges attention outputs between the two cores
    of a batch, then each core computes the final projection for its half
    of the tokens.
Host reassembles the full (4, 2048, 1024) output.
"""

import numpy as np

import concourse.bass as bass
import concourse.mybir as mybir
import concourse.tile as tile
from concourse import bacc, bass_utils
from concourse.bass import ds

N_CORES = 8
B, T, D, H = 4, 2048, 1024, 16
HD = D // H  # 64
FH = 512  # features per core (8 heads)
NFG = 4  # feature groups of 128 (2 heads each) per core
NTCH = 4  # 512-token chunks
NDS = 8  # 128-row contraction sub-tiles of D
NQC = 4  # 512-query chunks
NTT = 16  # 128-token tiles
F16 = mybir.dt.float16
F32 = mybir.dt.float32
EXP_SCALE = float(1.0 / np.sqrt(HD))


def build_nc(sim_mode=False, phase="full"):
    nc = bacc.Bacc("TRN2", target_bir_lowering=False, debug=False, num_devices=N_CORES)

    xT_d = nc.dram_tensor("xT", (D, T), F16, kind="ExternalInput")
    wq_d = nc.dram_tensor("wq", (D, FH), F16, kind="ExternalInput")
    wk_d = nc.dram_tensor("wk", (D, FH), F16, kind="ExternalInput")
    wv_d = nc.dram_tensor("wv", (D, FH), F16, kind="ExternalInput")
    wo_d = nc.dram_tensor("wo", (D, D), F16, kind="ExternalInput")
    bq_d = nc.dram_tensor("bq", (NFG, 128, 1), F32, kind="ExternalInput")
    bk_d = nc.dram_tensor("bk", (NFG, 128, 1), F32, kind="ExternalInput")
    bv_d = nc.dram_tensor("bv", (NFG, 128, 1), F32, kind="ExternalInput")
    bo_d = nc.dram_tensor("bo", (8, 128, 1), F32, kind="ExternalInput")
    mask_d = nc.dram_tensor("mask4", (128, 2048), F16, kind="ExternalInput")
    id_d = nc.dram_tensor("ident", (128, 128), F16, kind="ExternalInput")
    out_d = nc.dram_tensor("out_T", (D, T // 2), F32, kind="ExternalOutput")

    with tile.TileContext(nc) as tc:
        with (
            tc.tile_pool(name="const", bufs=1) as cpool,
            tc.tile_pool(name="ofeat", bufs=4) as opool,
            tc.tile_pool(name="psA", bufs=2, space="PSUM") as psA,
            tc.tile_pool(name="psS", bufs=2, space="PSUM") as psS,
            tc.tile_pool(name="psO", bufs=2, space="PSUM") as psO,
            tc.tile_pool(name="dram", bufs=1, space="DRAM") as dram,
        ):
            mask4 = cpool.tile([128, 2048], F16, tag="mask")
            nc.sync.dma_start(mask4[:], mask_d[:])
            ident = cpool.tile([128, 128], F16, tag="ident")
            nc.sync.dma_start(ident[:], id_d[:])
            bqs, bks, bvs, bos = [], [], [], []
            for i in range(NFG):
                bqt = cpool.tile([128, 1], F32, tag=f"bq{i}")
                nc.sync.dma_start(bqt[:], bq_d[i])
                bqs.append(bqt)
                bkt = cpool.tile([128, 1], F32, tag=f"bk{i}")
                nc.sync.dma_start(bkt[:], bk_d[i])
                bks.append(bkt)
                bvt = cpool.tile([128, 1], F32, tag=f"bv{i}")
                nc.sync.dma_start(bvt[:], bv_d[i])
                bvs.append(bvt)
            for i in range(8):
                bot = cpool.tile([128, 1], F32, tag=f"bo{i}")
                nc.sync.dma_start(bot[:], bo_d[i])
                bos.append(bot)

            # O_feat: per-fg [128 feat, 2048 tok] fp16, feature-major
            o_feat = []
            probe_srcs = []
            if phase != "qkv":
                for fg in range(NFG):
                    of = opool.tile([128, T], F16, tag="ofeat")
                    o_feat.append(of)

            with (
                tc.tile_pool(name="wqkv", bufs=1) as wpool,
                tc.tile_pool(name="xt", bufs=8) as xpool,
                tc.tile_pool(name="qk", bufs=2) as qkpool,
                tc.tile_pool(name="vst", bufs=36) as vpool,
                tc.tile_pool(name="vstg", bufs=4) as vstgpool,
                tc.tile_pool(name="pp", bufs=18) as ppool,
                tc.tile_pool(name="otok", bufs=4) as otokpool,
                tc.tile_pool(name="misc", bufs=8) as mpool,
            ):
                # resident xT: 8 tiles [128 d, 2048 t]
                xts = []
                for dsub in range(NDS):
                    xt = xpool.tile([128, T], F16, tag="xt")
                    nc.sync.dma_start(xt[:], xT_d[128 * dsub : 128 * (dsub + 1), :])
                    xts.append(xt)
                # resident weights: per proj 8 tiles [128 d, 512 f]
                wts = {}
                for pname, wd in (("q", wq_d), ("k", wk_d), ("v", wv_d)):
                    for dsub in range(NDS):
                        wt = wpool.tile([128, FH], F16, tag=f"w{pname}{dsub}")
                        nc.sync.dma_start(
                            wt[:], wd[128 * dsub : 128 * (dsub + 1), :]
                        )
                        wts[(pname, dsub)] = wt

                for fg in range(NFG):
                    f0 = 128 * fg  # feature offset within this core's 512
                    # ---- Q/K projections (feature-major [128 f, 2048 t]) ----
                    qf = qkpool.tile([128, T], F16, tag="qf")
                    kf = qkpool.tile([128, T], F16, tag="kf")
                    for pname, dst, bias in (("q", qf, bqs[fg]), ("k", kf, bks[fg])):
                        for tch in range(NTCH):
                            t0 = 512 * tch
                            ps = psA.tile([128, 512], F32, tag="proj")
                            for dsub in range(NDS):
                                nc.tensor.matmul(
                                    ps[:],
                                    wts[(pname, dsub)][:, f0 : f0 + 128],
                                    xts[dsub][:, t0 : t0 + 512],
                                    start=(dsub == 0),
                                    stop=(dsub == NDS - 1),
                                )
                            nc.vector.tensor_scalar_add(
                                dst[:, t0 : t0 + 512], ps[:], bias[:]
                            )
                    # ---- V projection -> token-major [128 t, 130] per t-tile ----
                    # cols: [head0 v(64) | 1.0 | head1 v(64) | 1.0]
                    vstore = []
                    for tt in range(NTT):
                        vt = vpool.tile([128, 130], F16, tag="vst")
                        nc.vector.memset(vt[:], 1.0)
                        vstore.append(vt)
                    for tch in range(NTCH):
                        t0 = 512 * tch
                        ps = psA.tile([128, 512], F32, tag="proj")
                        for dsub in range(NDS):
                            nc.tensor.matmul(
                                ps[:],
                                wts[("v", dsub)][:, f0 : f0 + 128],
                                xts[dsub][:, t0 : t0 + 512],
                                start=(dsub == 0),
                                stop=(dsub == NDS - 1),
                            )
                        vstg = vstgpool.tile([128, 512], F16, tag="vstg")
                        nc.vector.tensor_scalar_add(vstg[:], ps[:], bvs[fg][:])
                        for i in range(4):
                            tt = 4 * tch + i
                            pst = psA.tile([128, 128], F16, tag="proj")
                            nc.tensor.transpose(
                                pst[:], vstg[:, 128 * i : 128 * (i + 1)], ident[:]
                            )
                            # strided copy into [2 heads @ 65, 64] layout
                            nc.vector.tensor_copy(
                                vstore[tt][:].rearrange("p (h c) -> p h c", h=2)[
                                    :, :, 0:64
                                ],
                                pst[:].rearrange("p (h c) -> p h c", h=2),
                            )

                    # ---- attention for the 2 heads of this fg ----
                    if phase == "qkv":
                        probe_srcs.append((qf, kf, vstore[15]))
                        continue
                    # S_T in groups of 2 kblocks ([128, 1024] psum, double
                    # buffered) so the S-matmul stream runs ahead of exp.
                    # PV accumulates BOTH heads into one [128, 130] bank;
                    # normalize is one strided recip + one stride-0-broadcast
                    # multiply per query tile.
                    for j in range(NQC):
                        q0 = 512 * j
                        p_tiles = {}  # (hl, grp of 2 kblocks) -> [128,1024] f16
                        for hl in range(2):
                            frow = 64 * hl
                            for grp in range(2 * (j + 1)):
                                pss = psS.tile([128, 1024], F32, tag="s")
                                for ki in range(2):
                                    kb = 2 * grp + ki
                                    nc.tensor.matmul(
                                        pss[:, 512 * ki : 512 * (ki + 1)],
                                        kf[frow : frow + 64, 128 * kb : 128 * (kb + 1)],
                                        qf[frow : frow + 64, q0 : q0 + 512],
                                        start=True,
                                        stop=True,
                                    )
                                pt = ppool.tile([128, 1024], F16, tag="p")
                                nc.scalar.activation(
                                    pt[:],
                                    pss[:],
                                    mybir.ActivationFunctionType.Exp,
                                    scale=EXP_SCALE,
                                )
                                if grp >= 2 * j:  # diagonal groups (idle Pool engine)
                                    d = grp - 2 * j
                                    nc.gpsimd.tensor_mul(
                                        pt[:], pt[:], mask4[:, 1024 * d : 1024 * (d + 1)]
                                    )
                                p_tiles[(hl, grp)] = pt
                        for i in range(4):
                            qt = 4 * j + i
                            pso = psO.tile([128, 130], F32, tag="o")
                            nkb = 4 * j + i
                            for hl in range(2):
                                for kb in range(nkb + 1):
                                    grp, ki = kb // 2, kb % 2
                                    c0 = 512 * ki + 128 * i
                                    nc.tensor.matmul(
                                        pso[:, 65 * hl : 65 * hl + 65],
                                        p_tiles[(hl, grp)][:, c0 : c0 + 128],
                                        vstore[kb][:, 65 * hl : 65 * hl + 65],
                                        start=(kb == 0),
                                        stop=(kb == nkb),
                                    )
                            psv = pso[:].rearrange("p (h c) -> p h c", h=2)
                            rec = mpool.tile([128, 2], F32, tag="rec")
                            nc.vector.reciprocal(rec[:], psv[:, :, 64])
                            ot = otokpool.tile([128, 128], F16, tag="otok")
                            rec_b = bass.AP(
                                rec[:].tensor, rec[:].offset,
                                [rec[:].ap[0], [1, 2], [0, 64]],
                            )
                            nc.vector.tensor_tensor(
                                ot[:].rearrange("p (h c) -> p h c", h=2),
                                psv[:, :, 0:64],
                                rec_b,
                                mybir.AluOpType.mult,
                            )
                            pst = psA.tile([128, 128], F16, tag="proj")
                            nc.tensor.transpose(pst[:], ot[:], ident[:])
                            nc.vector.tensor_copy(
                                o_feat[fg][:, 128 * qt : 128 * (qt + 1)], pst[:]
                            )

            if phase in ("qkv", "attn"):
                with tc.tile_pool(name="probe", bufs=1) as prpool:
                    pr = prpool.tile([128, 512], F32, tag="pr")
                    if phase == "qkv":
                        q_, k_, v_ = probe_srcs[-1]
                        nc.vector.tensor_copy(pr[:, 0:128], q_[:, 0:128])
                        nc.vector.tensor_copy(pr[:, 128:256], k_[:, 0:128])
                        nc.vector.tensor_copy(pr[:, 256:321], v_[:, 0:65])
                    else:
                        nc.vector.tensor_copy(pr[:], o_feat[0][:, 0:512])
                    nc.sync.dma_start(out_d[0:128, 0:512], pr[:])
                nc.compile()
                return nc

            # ---- exchange: pairwise AllGather of full O_feat ----
            cc_in = dram.tile([FH, T], F16)
            cc_out = dram.tile([2, FH, T], F16)
            for fg in range(NFG):
                nc.sync.dma_start(cc_in[128 * fg : 128 * (fg + 1), :], o_feat[fg][:])
            if sim_mode:
                nc.sync.dma_start(cc_out[0], cc_in[:])
                nc.sync.dma_start(cc_out[1], cc_in[:])
                poff = 0
            else:
                nc.gpsimd.collective_compute(
                    "AllGather",
                    mybir.AluOpType.bypass,
                    replica_groups=[[0, 1], [2, 3], [4, 5], [6, 7]],
                    ins=[cc_in.opt()],
                    outs=[cc_out.opt()],
                )
                pid = nc.gpsimd.partition_id()
                poff = (pid % 2) * (T // 2)

            with (
                tc.tile_pool(name="att", bufs=8) as apool,
                tc.tile_pool(name="wo", bufs=8) as wopool,
                tc.tile_pool(name="outs", bufs=4) as outpool,
            ):
                att = []
                for s in range(2):
                    for fg in range(NFG):
                        at = apool.tile([128, T // 2], F16, tag="att")
                        if sim_mode:
                            nc.gpsimd.dma_start(
                                at[:],
                                cc_out[s][128 * fg : 128 * (fg + 1), 0 : T // 2],
                            )
                        else:
                            nc.gpsimd.dma_start(
                                at[:],
                                cc_out[s][128 * fg : 128 * (fg + 1), ds(poff, T // 2)],
                            )
                        att.append(at)
                wos = []
                for fs in range(8):
                    wt = wopool.tile([128, D], F16, tag="wo")
                    nc.sync.dma_start(wt[:], wo_d[128 * fs : 128 * (fs + 1), :])
                    wos.append(wt)
                for dt_ in range(8):
                    for tch in range(2):
                        t0 = 512 * tch
                        ps = psA.tile([128, 512], F32, tag="proj")
                        for fs in range(8):
                            nc.tensor.matmul(
                                ps[:],
                                wos[fs][:, 128 * dt_ : 128 * (dt_ + 1)],
                                att[fs][:, t0 : t0 + 512],
                                start=(fs == 0),
                                stop=(fs == 7),
                            )
                        ob = outpool.tile([128, 512], F32, tag="ob")
                        nc.vector.tensor_scalar_add(ob[:], ps[:], bos[dt_][:])
                        nc.sync.dma_start(
                            out_d[128 * dt_ : 128 * (dt_ + 1), t0 : t0 + 512], ob[:]
                        )

    nc.compile()
    return nc


def _prep_inputs(x, Wq, bq, Wk, bk, Wv, bv, Wo, bo):
    """Build the 8 per-core input maps."""
    x = np.asarray(x)
    mask4 = np.zeros((128, 2048), dtype=np.float16)
    r = np.arange(128)[:, None]
    for i in range(4):
        c = np.arange(512)[None, :]
        mask4[:, 512 * i : 512 * (i + 1)] = (c >= 128 * i + r).astype(np.float16)
    ident = np.eye(128, dtype=np.float16)
    wo16 = np.asarray(Wo).astype(np.float16)
    bo_r = np.asarray(bo).astype(np.float32).reshape(8, 128, 1)

    in_maps = []
    for c in range(N_CORES):
        b = c // 2
        hs = (c % 2) * FH
        in_maps.append(
            {
                "xT": np.ascontiguousarray(x[b].T).astype(np.float16),
                "wq": np.asarray(Wq)[:, hs : hs + FH].astype(np.float16),
                "wk": np.asarray(Wk)[:, hs : hs + FH].astype(np.float16),
                "wv": np.asarray(Wv)[:, hs : hs + FH].astype(np.float16),
                "wo": wo16,
                "bq": np.asarray(bq)[hs : hs + FH].astype(np.float32).reshape(4, 128, 1),
                "bk": np.asarray(bk)[hs : hs + FH].astype(np.float32).reshape(4, 128, 1),
                "bv": np.asarray(bv)[hs : hs + FH].astype(np.float32).reshape(4, 128, 1),
                "bo": bo_r,
                "mask4": mask4,
                "ident": ident,
            }
        )
    return in_maps


_NC_CACHE = None


def kernel(x, Wq, bq, Wk, bk, Wv, bv, Wo, bo):
    global _NC_CACHE
    if _NC_CACHE is None:
        _NC_CACHE = build_nc()
    nc = _NC_CACHE
    in_maps = _prep_inputs(x, Wq, bq, Wk, bk, Wv, bv, Wo, bo)
    res = bass_utils.run_bass_kernel_spmd(nc, in_maps, core_ids=list(range(N_CORES)))
    out = np.empty((B, T, D), dtype=np.float32)
    for c in range(N_CORES):
        b = c // 2
        half = c % 2
        out[b, half * (T // 2) : (half + 1) * (T // 2), :] = res.results[c]["out_T"].T
    return out
